# revision 4
# baseline (speedup 1.0000x reference)
"""Sparse window attention (8x8 windows, 8 heads, d=64) on 8 TRN2 NeuronCores.

Data-parallel: 1024 windows sharded 128/core. Per core, blocks of 512 tokens
(8 windows):
  x [512,512] --PE transpose--> xT --f32r matmuls--> qkT (feature-major, bf16)
  and v (token-major, bf16). Attention per (window-pair, head) as dense
  [64,128]x[64,128] bf16 matmuls (2 windows share the d-contraction rows; the
  off-diagonal window-cross blocks are computed but ignored). Bias added in
  PSUM via an identity matmul; exp+row-sum fused on ACT via accum_out;
  normalize on DVE; PE-transpose of attn; AV with token-major v as stationary;
  f32r output projection + bias; DMA out.
"""

import sys

sys.path.insert(0, "/opt/trn_rl_repo")

import numpy as np

import concourse.bass as bass
import concourse.mybir as mybir
import concourse.tile as tile
from concourse import bacc
from concourse.bass_utils import run_bass_kernel_spmd

F32 = mybir.dt.float32
F32R = mybir.dt.float32r
BF16 = mybir.dt.bfloat16

N_CORES = 8
TOK_PER_CORE = 8192          # 128 windows * 64 tokens
BLOCKS = 16                  # blocks of 512 tokens per core
EXP = mybir.ActivationFunctionType.Exp


def build():
    nc = bacc.Bacc(None, target_bir_lowering=False)

    x = nc.declare_dram_parameter("x", [TOK_PER_CORE, 512], F32, isOutput=False)
    wqk = nc.declare_dram_parameter("wqk", [512, 1024], F32, isOutput=False)
    wv = nc.declare_dram_parameter("wv", [512, 512], F32, isOutput=False)
    wo = nc.declare_dram_parameter("wo", [512, 512], F32, isOutput=False)
    bout = nc.declare_dram_parameter("bout", [128, 512], F32, isOutput=False)
    bias2 = nc.declare_dram_parameter("bias2", [128, 128], F32, isOutput=False)
    identf = nc.declare_dram_parameter("identf", [128, 128], F32, isOutput=False)
    out = nc.declare_dram_parameter("out", [TOK_PER_CORE, 512], F32, isOutput=True)

    x_r = x.rearrange("(blk tc p) c -> blk p tc c", tc=4, p=128)
    out_r = out.rearrange("(blk tc p) o -> blk tc p o", tc=4, p=128)

    with tile.TileContext(nc) as tc:
        with (
            tc.tile_pool(name="const", bufs=1) as const,
            tc.tile_pool(name="xin", bufs=2) as xin,
            tc.tile_pool(name="xt", bufs=2) as xt,
            tc.tile_pool(name="qk", bufs=2) as qkp,
            tc.tile_pool(name="vv", bufs=2) as vvp,
            tc.tile_pool(name="ot", bufs=2) as otp,
            tc.tile_pool(name="fin", bufs=3) as finp,
            tc.tile_pool(name="att", bufs=3) as attp,
            tc.tile_pool(name="attn2", bufs=3) as attn2p,
            tc.tile_pool(name="attt", bufs=3) as atttp,
            tc.tile_pool(name="small", bufs=6) as smallp,
            tc.tile_pool(name="ps_big", bufs=2, space="PSUM") as ps_big,
            tc.tile_pool(name="ps_sim", bufs=2, space="PSUM") as ps_sim,
            tc.tile_pool(name="ps_tr", bufs=2, space="PSUM") as ps_tr,
            tc.tile_pool(name="ps_av", bufs=2, space="PSUM") as ps_av,
        ):
            # ---- constants ----
            wqk_sb = const.tile([128, 4, 1024], F32)
            nc.sync.dma_start(wqk_sb, wqk.rearrange("(cc p) f -> p cc f", p=128))
            wv_sb = const.tile([128, 4, 512], F32)
            nc.sync.dma_start(wv_sb, wv.rearrange("(cc p) f -> p cc f", p=128))
            wo_sb = const.tile([128, 4, 512], F32)
            nc.sync.dma_start(wo_sb, wo.rearrange("(fc p) o -> p fc o", p=128))
            bout_sb = const.tile([128, 512], F32)
            nc.sync.dma_start(bout_sb, bout[:, :])
            bias2_f = const.tile([128, 128], F32)
            nc.sync.dma_start(bias2_f, bias2[:, :])
            identf_sb = const.tile([128, 128], F32)
            nc.sync.dma_start(identf_sb, identf[:, :])
            bias2_bf = const.tile([128, 128], BF16)
            nc.scalar.copy(bias2_bf, bias2_f)
            identb_sb = const.tile([128, 128], BF16)
            nc.scalar.copy(identb_sb, identf_sb)
            wqk_r = const.tile([128, 4, 1024], F32R)
            nc.vector.tensor_copy(wqk_r, wqk_sb)
            wv_r = const.tile([128, 4, 512], F32R)
            nc.vector.tensor_copy(wv_r, wv_sb)
            wo_r = const.tile([128, 4, 512], F32R)
            nc.vector.tensor_copy(wo_r, wo_sb)

            for b in range(BLOCKS):
                # ---- load x, transpose to xT ----
                x_sb = xin.tile([128, 4, 512], F32)
                nc.sync.dma_start(x_sb, x_r[b])
                xT_sb = xt.tile([128, 4, 512], F32R)
                for tc_i in range(4):
                    for cc in range(4):
                        pst = ps_tr.tile([128, 128], F32, tag="tr")
                        nc.tensor.matmul(
                            pst,
                            lhsT=x_sb[:, tc_i, cc * 128 : (cc + 1) * 128],
                            rhs=identf_sb,
                            is_transpose=True,
                        )
                        nc.vector.tensor_copy(
                            xT_sb[:, cc, tc_i * 128 : (tc_i + 1) * 128], pst
                        )

                # ---- q,k feature-major [f, t] (bf16), SCALE folded in host ----
                qkT_sb = qkp.tile([128, 8, 512], BF16)
                for m in range(8):
                    ps = ps_big.tile([128, 512], F32, tag="big")
                    for cc in range(4):
                        nc.tensor.matmul(
                            ps,
                            lhsT=wqk_r[:, cc, m * 128 : (m + 1) * 128],
                            rhs=xT_sb[:, cc, :],
                            start=(cc == 0),
                            stop=(cc == 3),
                        )
                    nc.scalar.copy(qkT_sb[:, m, :], ps)

                # ---- v token-major [t, f] (bf16) ----
                v_sb = vvp.tile([128, 4, 512], BF16)
                for tc_i in range(4):
                    ps = ps_big.tile([128, 512], F32, tag="big")
                    for cc in range(4):
                        nc.tensor.matmul(
                            ps,
                            lhsT=xT_sb[:, cc, tc_i * 128 : (tc_i + 1) * 128],
                            rhs=wv_r[:, cc, :],
                            start=(cc == 0),
                            stop=(cc == 3),
                        )
                    nc.scalar.copy(v_sb[:, tc_i, :], ps)

                # ---- attention: window pairs x heads ----
                outT_sb = otp.tile([128, 4, 512], F32R)
                for wp in range(4):
                    for h in range(8):
                        r0 = (h % 2) * 64
                        mq = h // 2
                        qh = qkT_sb[r0 : r0 + 64, mq, wp * 128 : (wp + 1) * 128]
                        kh = qkT_sb[r0 : r0 + 64, 4 + mq, wp * 128 : (wp + 1) * 128]
                        ps = ps_sim.tile([128, 128], F32)
                        nc.tensor.matmul(ps, lhsT=qh, rhs=kh, start=True, stop=False)
                        nc.tensor.matmul(
                            ps, lhsT=identb_sb, rhs=bias2_bf, start=False, stop=True
                        )
                        att_u = attp.tile([128, 128], BF16)
                        den = smallp.tile([128, 1], F32, tag="den")
                        rec = smallp.tile([128, 1], F32, tag="rec")
                        nc.scalar.activation(
                            att_u[0:64, 0:64], ps[0:64, 0:64], EXP,
                            accum_out=den[0:64, :],
                        )
                        nc.scalar.activation(
                            att_u[64:128, 64:128], ps[64:128, 64:128], EXP,
                            accum_out=den[64:128, :],
                        )
                        nc.vector.reciprocal(rec, den)
                        attn_n = attn2p.tile([128, 128], BF16)
                        nc.gpsimd.memset(attn_n[0:64, 64:128], 0.0)
                        nc.gpsimd.memset(attn_n[64:128, 0:64], 0.0)
                        nc.vector.tensor_scalar_mul(
                            attn_n[0:64, 0:64], att_u[0:64, 0:64], rec[0:64, :]
                        )
                        nc.vector.tensor_scalar_mul(
                            attn_n[64:128, 64:128], att_u[64:128, 64:128],
                            rec[64:128, :],
                        )
                        pst = ps_tr.tile([128, 128], BF16, tag="tr")
                        nc.tensor.transpose(pst, attn_n, identb_sb)
                        attT = atttp.tile([128, 128], BF16)
                        nc.vector.tensor_copy(attT, pst)
                        pso = ps_av.tile([64, 128], F32)
                        nc.tensor.matmul(
                            pso,
                            lhsT=v_sb[:, wp, h * 64 : (h + 1) * 64],
                            rhs=attT,
                            start=True,
                            stop=True,
                        )
                        nc.scalar.copy(
                            outT_sb[r0 : r0 + 64, mq, wp * 128 : (wp + 1) * 128], pso
                        )

                # ---- output projection + bias ----
                for tc_i in range(4):
                    psf = ps_big.tile([128, 512], F32, tag="big")
                    for fc in range(4):
                        nc.tensor.matmul(
                            psf,
                            lhsT=outT_sb[:, fc, tc_i * 128 : (tc_i + 1) * 128],
                            rhs=wo_r[:, fc, :],
                            start=(fc == 0),
                            stop=(fc == 3),
                        )
                    fin = finp.tile([128, 512], F32)
                    nc.vector.tensor_add(fin, psf, bout_sb)
                    nc.sync.dma_start(out_r[b, tc_i], fin)

    nc.finalize()
    return nc


_NC_CACHE = None


def _get_nc():
    global _NC_CACHE
    if _NC_CACHE is None:
        _NC_CACHE = build()
    return _NC_CACHE


def _host_prep(x, w_qkv, w_out, b_out, Mat):
    x = np.asarray(x, np.float32)
    w_qkv = np.asarray(w_qkv, np.float32)
    w_out = np.asarray(w_out, np.float32)
    b_out = np.asarray(b_out, np.float32)
    Mat = np.asarray(Mat, np.float32)

    g = np.arange(8)
    b1, b2 = np.meshgrid(g, g, indexing="ij")
    coords = np.stack([b1.ravel(), b2.ravel()], axis=1)
    rel = coords[None, :, :] - coords[:, None, :] + 7
    bidx = rel[:, :, 0] * 15 + rel[:, :, 1]
    bias = Mat[bidx]                                   # [64, 64]
    bias2 = np.ascontiguousarray(np.tile(bias, (2, 2)).astype(np.float32))

    wqkvT = np.ascontiguousarray(w_qkv.T).astype(np.float32)   # [512, 1536]
    wqkvT[:, :512] *= 64 ** -0.5
    wqk = np.ascontiguousarray(wqkvT[:, :1024])
    wv = np.ascontiguousarray(wqkvT[:, 1024:])
    wo = np.ascontiguousarray(w_out.T).astype(np.float32)
    bout = np.ascontiguousarray(np.tile(b_out[None, :], (128, 1)))
    identf = np.eye(128, dtype=np.float32)
    xf = np.ascontiguousarray(x.reshape(1024 * 64, 512))

    in_maps = []
    for c in range(N_CORES):
        in_maps.append(
            {
                "x": np.ascontiguousarray(
                    xf[c * TOK_PER_CORE : (c + 1) * TOK_PER_CORE]
                ),
                "wqk": wqk,
                "wv": wv,
                "wo": wo,
                "bout": bout,
                "bias2": bias2,
                "identf": identf,
            }
        )
    return in_maps


def run(inputs, trace=False):
    nc = _get_nc()
    in_maps = _host_prep(**inputs)
    res = run_bass_kernel_spmd(
        nc, in_maps, core_ids=list(range(N_CORES)), trace=trace
    )
    out = np.concatenate([res.results[c]["out"] for c in range(N_CORES)], axis=0)
    return out.reshape(1024, 64, 512), res


def kernel(x, w_qkv, w_out, b_out, Mat):
    out, _ = run(dict(x=x, w_qkv=w_qkv, w_out=w_out, b_out=b_out, Mat=Mat))
    return out
